# revision 1
# baseline (speedup 1.0000x reference)
"""Trainium2 Bass kernel for nn_Head (single-head causal self-attention).

Module:  q = x@Wq.T, k = x@Wk.T, v = x@Wv.T
         wei = softmax(causal_mask(q@k.T * E**-0.5))
         out = wei @ v
Shapes:  x [2048, 128, 192], Wq/Wk/Wv [192, 192] -> out [2048, 128, 192]

Strategy (pure data parallel over the batch dim, 8 cores x 256 batches):
  - Weight fold: wei = x @ A @ x.T with A = (Wq.T @ Wk) * SCALE, so only one
    projection ("g = x @ A") is needed for the attention logits.
  - Host prepares x transposed per-core as xt[e, b*T + t] in bf16 (layout +
    dtype prep only; all model FLOPs run on device).
  - Per batch on device:  gT = A.T @ xT (A-stationary, 4-batch column blocks),
    wei = gT.T @ xT, P = exp(wei) (ACT), Pm = P*mask with row-sum (DVE TTR),
    Pm *= 1/s, PT = transpose(Pm) (PE), v = xT.T @ Wv.T, o = PT.T @ v.
"""

import os
import sys

sys.path.insert(0, "/opt/trn_rl_repo")

import numpy as np
import ml_dtypes
from contextlib import ExitStack

import json

import concourse.bass as bass
import concourse.bass2jax as bass2jax
import concourse.mybir as mybir
import concourse.tile as tile
from concourse.bass_utils import (
    compile_bir_kernel as _orig_compile_bir_kernel,
    run_bass_kernel_spmd,
)

BF16 = mybir.dt.bfloat16
F32 = mybir.dt.float32
NPBF16 = ml_dtypes.bfloat16

B, T, E, H = 2048, 128, 192, 192
NCORES = 8
NB = B // NCORES            # batches per core
SCALE = float(E) ** -0.5
G = 8                       # batches per DMA group
QUAD = 4                    # batches sharing one PSUM bank for wei/PT
NGROUPS = NB // G


def _patch_tile_tail_drain():
    """Walrus rejects the TileContext tail Drain when it carries more than a
    couple of sem waits ("Too many sync wait commands").  Redistribute the
    waits onto single-wait SP nops emitted between the drain and barrier."""
    if getattr(tile.TileContext, "_tail_drain_patched", False):
        return

    def _drain_and_barrier(self, tick_clock, wait_clock):
        from concourse.tile import ScopedClock

        drain_inst = self.nc.sync.drain()
        wait_clock.add_sem_waits(
            drain_inst.ins, ScopedClock({None: tick_clock.global_clock})
        )
        waits = list(drain_inst.ins.sync_info.on_wait or [])
        if len(waits) > 1:
            drain_inst.ins.sync_info = mybir.SyncInfo(
                on_wait=[waits[0]], on_update=[]
            )
            for w in waits[1:]:
                nop = self.nc.sync.nop()
                nop.ins.sync_info = mybir.SyncInfo(on_wait=[w], on_update=[])
        self.nc.all_engine_barrier()
        assert self.sems is not None
        popped = self.nc._tile_sem_poison_stack.pop()
        assert popped is self._sem_poison
        self.nc.clear_and_free_semaphores(list(self.sems.allocated().values()))
        self.nc.all_engine_barrier()

    tile.TileContext._drain_and_barrier = _drain_and_barrier
    tile.TileContext._tail_drain_patched = True


def _split_multi_waits(bir_json: bytes) -> bytes:
    """This container's walrus supports only ONE sync-wait slot per
    instruction ("Too many sync wait commands").  Hoist extra waits onto
    single-wait NoOps inserted just before the instruction (same engine, so
    per-engine program order and blocking semantics are preserved)."""
    d = json.loads(bir_json)
    n = 0
    for f in d.get("functions", []):
        for bb in f.get("blocks", []):
            insts = bb.get("instructions", [])
            out = []
            changed = False
            for inst in insts:
                si = inst.get("sync_info")
                waits = (si.get("on_wait") or []) if si else []
                if len(waits) > 1:
                    changed = True
                    for w in waits[:-1]:
                        n += 1
                        out.append({
                            "debug": inst.get("debug"),
                            "engine": inst["engine"],
                            "ins": [],
                            "name": f"WSPLIT-{n}",
                            "opcode": "NoOp",
                            "outs": [],
                            "sync_info": {"on_update": [], "on_wait": [w]},
                        })
                    si["on_wait"] = [waits[-1]]
                out.append(inst)
            if changed:
                bb["instructions"] = out
    if n == 0:
        return bir_json
    return json.dumps(d).encode()


def _patched_compile_bir_kernel(bir_json, tmpdir, neff_name="file.neff"):
    if isinstance(bir_json, str):
        bir_json = bir_json.encode()
    return _orig_compile_bir_kernel(_split_multi_waits(bir_json), tmpdir, neff_name)


bass2jax.compile_bir_kernel = _patched_compile_bir_kernel


def build_nc(nb=NB):
    _patch_tile_tail_drain()
    nc = bass.Bass(trn_type="TRN2")

    xt = nc.dram_tensor("xt", [E, nb * T], BF16, kind="ExternalInput")
    a = nc.dram_tensor("a", [E, E], BF16, kind="ExternalInput")
    wvt = nc.dram_tensor("wvt", [E, H], BF16, kind="ExternalInput")
    o = nc.dram_tensor("o", [nb, T, H], F32, kind="ExternalOutput")

    ngroups = nb // G
    mult = mybir.AluOpType.mult
    add = mybir.AluOpType.add

    with tile.TileContext(nc) as tc, ExitStack() as ctx:
        singles = ctx.enter_context(tc.tile_pool(name="singles", bufs=1))
        px = ctx.enter_context(tc.tile_pool(name="px", bufs=3))
        pgsb = ctx.enter_context(tc.tile_pool(name="pgsb", bufs=2))
        pp = ctx.enter_context(tc.tile_pool(name="pp", bufs=3))
        psr = ctx.enter_context(tc.tile_pool(name="psr", bufs=4))
        pptsb = ctx.enter_context(tc.tile_pool(name="pptsb", bufs=3))
        pvsb = ctx.enter_context(tc.tile_pool(name="pvsb", bufs=6))
        posb = ctx.enter_context(tc.tile_pool(name="posb", bufs=3))

        pglo = ctx.enter_context(tc.tile_pool(name="pglo", bufs=1, space="PSUM"))
        pghi = ctx.enter_context(tc.tile_pool(name="pghi", bufs=1, space="PSUM"))
        pw = ctx.enter_context(tc.tile_pool(name="pw", bufs=1, space="PSUM"))
        ppt = ctx.enter_context(tc.tile_pool(name="ppt", bufs=1, space="PSUM"))
        pv = ctx.enter_context(tc.tile_pool(name="pv", bufs=2, space="PSUM"))
        po = ctx.enter_context(tc.tile_pool(name="po", bufs=2, space="PSUM"))

        # Constants: A (lhsT for gT), WvT (rhs for v), identity, causal mask.
        a_lo = singles.tile([128, E], BF16, tag="a_lo")
        a_hi = singles.tile([64, E], BF16, tag="a_hi")
        nc.sync.dma_start(out=a_lo, in_=a[0:128, :])
        nc.sync.dma_start(out=a_hi, in_=a[128:192, :])
        wvt_lo = singles.tile([128, H], BF16, tag="wvt_lo")
        wvt_hi = singles.tile([64, H], BF16, tag="wvt_hi")
        nc.sync.dma_start(out=wvt_lo, in_=wvt[0:128, :])
        nc.sync.dma_start(out=wvt_hi, in_=wvt[128:192, :])

        ident = singles.tile([128, 128], BF16, tag="ident")
        nc.gpsimd.memset(ident, 0.0)
        nc.gpsimd.affine_select(
            out=ident, in_=ident,
            compare_op=mybir.AluOpType.not_equal,
            fill=1.0, base=0, pattern=[[-1, 128]], channel_multiplier=1,
        )
        # mask4[q, g, k] = 1.0 if k <= q else 0.0  (causal mask, tiled QUAD x)
        mask4 = singles.tile([128, QUAD, 128], BF16, tag="mask4")
        nc.gpsimd.memset(mask4, 1.0)
        nc.gpsimd.affine_select(
            out=mask4, in_=mask4,
            compare_op=mybir.AluOpType.is_ge,
            fill=0.0, base=0, pattern=[[0, QUAD], [-1, 128]], channel_multiplier=1,
        )

        # Software pipeline over quads: at iteration Q emit
        #   gT(Q), v(Q)  ->  PT(Q-1)  ->  wei(Q)  ->  o(Q-2)
        # so PE never waits on the vector-side chain exp -> mask -> copy.
        nq = nb // QUAD
        x_tiles = {}     # group -> (xlo, xhi)
        gsb_t = {}       # Q -> (gsb_lo, gsb_hi)
        pm_t = {}        # Q -> pm
        ptsb_t = {}      # Q -> pt_sb
        vsb_t = {}       # Q -> [v_sb pair0, v_sb pair1]
        osb_t = {}       # group -> o_sb

        for Q in range(nq + 2):
            if Q < nq:
                g = Q * QUAD // G
                if (Q * QUAD) % G == 0:
                    gcol = g * G * T
                    xlo = px.tile([128, G * T], BF16, tag="xlo")
                    xhi = px.tile([64, G * T], BF16, tag="xhi")
                    nc.sync.dma_start(out=xlo, in_=xt[0:128, gcol : gcol + G * T])
                    nc.sync.dma_start(out=xhi, in_=xt[128:192, gcol : gcol + G * T])
                    x_tiles[g] = (xlo, xhi)
                xlo, xhi = x_tiles[g]
                qs = (Q * QUAD * T) % (G * T)
                qcols = slice(qs, qs + QUAD * T)

                # gT = A.T @ xT for 4 batches (N=512)
                glo = pglo.tile([128, QUAD * T], F32, tag="glo")
                ghi = pghi.tile([64, QUAD * T], F32, tag="ghi")
                nc.tensor.matmul(glo, a_lo[:, 0:128], xlo[:, qcols],
                                 start=True, stop=False)
                nc.tensor.matmul(glo, a_hi[:, 0:128], xhi[:, qcols],
                                 start=False, stop=True)
                nc.tensor.matmul(ghi, a_lo[:, 128:192], xlo[:, qcols],
                                 start=True, stop=False)
                nc.tensor.matmul(ghi, a_hi[:, 128:192], xhi[:, qcols],
                                 start=False, stop=True)
                gsb_lo = pgsb.tile([128, QUAD * T], BF16, tag="gsb_lo")
                gsb_hi = pgsb.tile([64, QUAD * T], BF16, tag="gsb_hi")
                nc.scalar.copy(out=gsb_lo, in_=glo)
                nc.vector.tensor_copy(out=gsb_hi, in_=ghi)
                gsb_t[Q] = (gsb_lo, gsb_hi)

                # v = xT.T @ WvT, two batches per PSUM bank; v_ext = [v | 1]
                vsb_t[Q] = []
                for pr in range(QUAD // 2):
                    v_ps = pv.tile([128, 2, H], F32, tag="v_ps")
                    for jj in range(2):
                        bs = qs + (pr * 2 + jj) * T
                        nc.tensor.matmul(v_ps[:, jj, :], xlo[:, bs : bs + T],
                                         wvt_lo, start=True, stop=False)
                        nc.tensor.matmul(v_ps[:, jj, :], xhi[:, bs : bs + T],
                                         wvt_hi, start=False, stop=True)
                    v_sb = pvsb.tile([128, 2, H + 8], BF16, tag="v_sb")
                    nc.scalar.copy(out=v_sb[:, :, 0:H], in_=v_ps)
                    nc.gpsimd.memset(v_sb[:, :, H : H + 1], 1.0)
                    vsb_t[Q].append(v_sb)

            # PT(Q-1) = transpose(Pm(Q-1))
            if 1 <= Q <= nq:
                pm = pm_t.pop(Q - 1)
                pt_ps = ppt.tile([128, QUAD, T], BF16, tag="pt_ps")
                for j in range(QUAD):
                    nc.tensor.transpose(pt_ps[:, j, :], pm[:, j, :], ident)
                pt_sb = pptsb.tile([128, QUAD, T], BF16, tag="pt_sb")
                nc.vector.tensor_copy(out=pt_sb, in_=pt_ps)
                ptsb_t[Q - 1] = pt_sb

            if Q < nq:
                # wei[j] = gT_j.T @ xT_j  (one PSUM bank per quad)
                gsb_lo, gsb_hi = gsb_t.pop(Q)
                wei = pw.tile([128, QUAD, T], F32, tag="wei")
                for j in range(QUAD):
                    bs = qs + j * T
                    jc = slice(j * T, (j + 1) * T)
                    nc.tensor.matmul(wei[:, j, :], gsb_lo[:, jc],
                                     xlo[:, bs : bs + T], start=True, stop=False)
                    nc.tensor.matmul(wei[:, j, :], gsb_hi[:, jc],
                                     xhi[:, bs : bs + T], start=False, stop=True)

                # P = exp(wei) (ACT) ; Pm = P * causal_mask (DVE)
                p_sb = pp.tile([128, QUAD, T], BF16, tag="p_sb")
                nc.scalar.activation(out=p_sb, in_=wei,
                                     func=mybir.ActivationFunctionType.Exp)
                pm = pp.tile([128, QUAD, T], BF16, tag="pm")
                nc.vector.tensor_mul(pm, p_sb, mask4)
                pm_t[Q] = pm

            # o(Q-2) = PT.T @ v_ext ; col H = softmax denominator
            if Q >= 2:
                oq = Q - 2
                gb = oq * QUAD // G
                ob0 = (oq * QUAD) % G
                if ob0 == 0:
                    osb_t[gb] = posb.tile([128, G, H], F32, tag="o_sb",
                                          name="o_sb")
                o_sb = osb_t[gb]
                pt_sb = ptsb_t.pop(oq)
                for pr in range(QUAD // 2):
                    v_sb = vsb_t[oq][pr]
                    o_ps = po.tile([128, 2, H + 8], F32, tag="o_ps")
                    for jj in range(2):
                        j = pr * 2 + jj
                        nc.tensor.matmul(o_ps[:, jj, 0 : H + 1], pt_sb[:, j, :],
                                         v_sb[:, jj, 0 : H + 1],
                                         start=True, stop=True)
                    r = psr.tile([128, 2], F32, tag="r")
                    nc.vector.reciprocal(out=r, in_=o_ps[:, :, H])
                    ob = ob0 + pr * 2
                    nc.vector.tensor_scalar_mul(
                        out=o_sb[:, ob, :], in0=o_ps[:, 0, 0:H],
                        scalar1=r[:, 0:1],
                    )
                    nc.scalar.mul(
                        out=o_sb[:, ob + 1, :], in_=o_ps[:, 1, 0:H],
                        mul=r[:, 1:2],
                    )
                del vsb_t[oq]
                if ob0 + QUAD == G:
                    nc.sync.dma_start(
                        out=o[gb * G : (gb + 1) * G, :, :].rearrange(
                            "b t h -> t b h"
                        ),
                        in_=o_sb,
                    )
                    del osb_t[gb]
    return nc


_cached = {}


def _get_nc(nb):
    if nb not in _cached:
        _cached[nb] = build_nc(nb)
    return _cached[nb]


def prep_inputs(x, Wq, Wk, Wv, nb=NB, ncores=NCORES):
    """Host-side sharding + layout/dtype prep + weight folding."""
    x = np.asarray(x, dtype=np.float32)
    A = (np.asarray(Wq, np.float32).T @ np.asarray(Wk, np.float32)) * SCALE
    a_bf = np.ascontiguousarray(A).astype(NPBF16)
    wvt_bf = np.ascontiguousarray(np.asarray(Wv, np.float32).T).astype(NPBF16)
    in_maps = []
    for c in range(ncores):
        shard = x[c * nb : (c + 1) * nb]                      # [nb, T, E]
        xt = np.ascontiguousarray(shard.transpose(2, 0, 1)).reshape(E, nb * T)
        in_maps.append({"xt": xt.astype(NPBF16), "a": a_bf, "wvt": wvt_bf})
    return in_maps


def kernel(x, Wq, Wk, Wv, _trace=False):
    nc = _get_nc(NB)
    in_maps = prep_inputs(x, Wq, Wk, Wv)
    res = run_bass_kernel_spmd(
        nc, in_maps, core_ids=list(range(NCORES)), trace=_trace
    )
    out = np.concatenate([res.results[c]["o"] for c in range(NCORES)], axis=0)
    if _trace:
        kernel.last_result = res
    return out



# revision 5
# speedup vs baseline: 1.1933x; 1.1933x over previous
"""Trainium2 Bass kernel for nn_Head (single-head causal self-attention).

Module:  q = x@Wq.T, k = x@Wk.T, v = x@Wv.T
         wei = softmax(causal_mask(q@k.T * E**-0.5))
         out = wei @ v
Shapes:  x [2048, 128, 192], Wq/Wk/Wv [192, 192] -> out [2048, 128, 192]

Strategy (pure data parallel over the batch dim, 8 cores x 256 batches):
  - Weight fold: wei = x @ A @ x.T with A = (Wq.T @ Wk) * SCALE, so only one
    projection ("g = x @ A") is needed for the attention logits.
  - Host prepares x transposed per-core as xt[e, b*T + t] in bf16.
  - Logits are computed TRANSPOSED (weiT[k, q] instead of wei[q, k]) so the
    masked exp'd tile is directly the lhsT of the output matmul -- no PE
    transposes at all.
  - The E=192 contraction of weiT is done as two independent single matmuls
    (K=128 and K=64) into two PSUM banks summed by one DVE op: single
    matmuls hide their LDWEIGHTS (~56 ns at N=128) while chained
    accumulation pairs pay ~+108 ns each on this platform.
  - Row sums ride along the output matmul via a ones-column on v; the
    normalization happens on ScalarE with per-partition reciprocal scalars.
  - Dense PE stream (software-pipelined by one/two quads) keeps the HAM
    clock gate at 2.4 GHz.
"""

import os
import sys

sys.path.insert(0, "/opt/trn_rl_repo")

import numpy as np
import ml_dtypes
from contextlib import ExitStack

import json

import concourse.bass as bass
import concourse.bass2jax as bass2jax
import concourse.mybir as mybir
import concourse.tile as tile
from concourse.bass_utils import (
    compile_bir_kernel as _orig_compile_bir_kernel,
    run_bass_kernel_spmd,
)

BF16 = mybir.dt.bfloat16
F32 = mybir.dt.float32
NPBF16 = ml_dtypes.bfloat16

B, T, E, H = 2048, 128, 192, 192
NCORES = 8
NB = B // NCORES            # batches per core
SCALE = float(E) ** -0.5
G = 8                       # batches per DMA group
QUAD = 4                    # batches per pipeline stage
NGROUPS = NB // G


def _patch_tile_tail_drain():
    """Walrus rejects the TileContext tail Drain when it carries more than a
    couple of sem waits ("Too many sync wait commands").  Redistribute the
    waits onto single-wait SP nops emitted between the drain and barrier."""
    if getattr(tile.TileContext, "_tail_drain_patched", False):
        return

    def _drain_and_barrier(self, tick_clock, wait_clock):
        from concourse.tile import ScopedClock

        drain_inst = self.nc.sync.drain()
        wait_clock.add_sem_waits(
            drain_inst.ins, ScopedClock({None: tick_clock.global_clock})
        )
        waits = list(drain_inst.ins.sync_info.on_wait or [])
        if len(waits) > 1:
            drain_inst.ins.sync_info = mybir.SyncInfo(
                on_wait=[waits[0]], on_update=[]
            )
            for w in waits[1:]:
                nop = self.nc.sync.nop()
                nop.ins.sync_info = mybir.SyncInfo(on_wait=[w], on_update=[])
        self.nc.all_engine_barrier()
        assert self.sems is not None
        popped = self.nc._tile_sem_poison_stack.pop()
        assert popped is self._sem_poison
        self.nc.clear_and_free_semaphores(list(self.sems.allocated().values()))
        self.nc.all_engine_barrier()

    tile.TileContext._drain_and_barrier = _drain_and_barrier
    tile.TileContext._tail_drain_patched = True


def _split_multi_waits(bir_json: bytes) -> bytes:
    """This container's walrus supports only ONE sync-wait slot per
    instruction ("Too many sync wait commands").  Hoist extra waits onto
    single-wait NoOps inserted just before the instruction (same engine, so
    per-engine program order and blocking semantics are preserved)."""
    d = json.loads(bir_json)
    n = 0
    for f in d.get("functions", []):
        for bb in f.get("blocks", []):
            insts = bb.get("instructions", [])
            out = []
            changed = False
            for inst in insts:
                si = inst.get("sync_info")
                waits = (si.get("on_wait") or []) if si else []
                if len(waits) > 1:
                    changed = True
                    for w in waits[:-1]:
                        n += 1
                        out.append({
                            "debug": inst.get("debug"),
                            "engine": inst["engine"],
                            "ins": [],
                            "name": f"WSPLIT-{n}",
                            "opcode": "NoOp",
                            "outs": [],
                            "sync_info": {"on_update": [], "on_wait": [w]},
                        })
                    si["on_wait"] = [waits[-1]]
                out.append(inst)
            if changed:
                bb["instructions"] = out
    if n == 0:
        return bir_json
    return json.dumps(d).encode()


def _patched_compile_bir_kernel(bir_json, tmpdir, neff_name="file.neff"):
    if isinstance(bir_json, str):
        bir_json = bir_json.encode()
    return _orig_compile_bir_kernel(_split_multi_waits(bir_json), tmpdir, neff_name)


bass2jax.compile_bir_kernel = _patched_compile_bir_kernel


def build_nc(nb=NB):
    _patch_tile_tail_drain()
    nc = bass.Bass(trn_type="TRN2")

    xt = nc.dram_tensor("xt", [E, nb * T], BF16, kind="ExternalInput")
    a = nc.dram_tensor("a", [E, E], BF16, kind="ExternalInput")
    wvt = nc.dram_tensor("wvt", [E, H], BF16, kind="ExternalInput")
    # Output laid out [t, b, h] for contiguous DMA; host transposes to
    # [b, t, h].
    o = nc.dram_tensor("o", [T, nb, H], F32, kind="ExternalOutput")

    nq = nb // QUAD
    Exp = mybir.ActivationFunctionType.Exp

    with tile.TileContext(nc) as tc, ExitStack() as ctx:
        singles = ctx.enter_context(tc.tile_pool(name="singles", bufs=1))
        px = ctx.enter_context(tc.tile_pool(name="px", bufs=3))
        pgsb = ctx.enter_context(tc.tile_pool(name="pgsb", bufs=3))
        ppa = ctx.enter_context(tc.tile_pool(name="ppa", bufs=2))
        ppb = ctx.enter_context(tc.tile_pool(name="ppb", bufs=2))
        ppm = ctx.enter_context(tc.tile_pool(name="ppm", bufs=3))
        pvsb = ctx.enter_context(tc.tile_pool(name="pvsb", bufs=6))
        psr = ctx.enter_context(tc.tile_pool(name="psr", bufs=4))
        posb = ctx.enter_context(tc.tile_pool(name="posb", bufs=2))

        pg = ctx.enter_context(tc.tile_pool(name="pg", bufs=1, space="PSUM"))
        pwa = ctx.enter_context(tc.tile_pool(name="pwa", bufs=1, space="PSUM"))
        pwb = ctx.enter_context(tc.tile_pool(name="pwb", bufs=1, space="PSUM"))
        pv = ctx.enter_context(tc.tile_pool(name="pv", bufs=2, space="PSUM"))
        po = ctx.enter_context(tc.tile_pool(name="po", bufs=2, space="PSUM"))

        # Constants: A (lhsT for gT), WvT (rhs for v).
        a_lo = singles.tile([128, E], BF16, tag="a_lo")
        a_hi = singles.tile([64, E], BF16, tag="a_hi")
        nc.sync.dma_start(out=a_lo, in_=a[0:128, :])
        nc.sync.dma_start(out=a_hi, in_=a[128:192, :])
        wvt_lo = singles.tile([128, H], BF16, tag="wvt_lo")
        wvt_hi = singles.tile([64, H], BF16, tag="wvt_hi")
        nc.sync.dma_start(out=wvt_lo, in_=wvt[0:128, :])
        nc.sync.dma_start(out=wvt_hi, in_=wvt[128:192, :])

        # Software pipeline over quads: iteration Q emits
        #   gT(Q), v(Q)  ->  weiT(Q-1) + exp + mask  ->  o(Q-2)
        x_tiles = {}     # group -> (xlo, xhi)
        gsb_t = {}       # Q -> gsb ([128,1024]: gT_lo | gT_hi)
        pm_t = {}        # Q -> masked exp'd weiT (PmT)
        vsb_t = {}       # (Q, pr) -> v_sb pair
        osb_t = {}       # group -> o_sb

        for Q in range(nq + 2):
            if Q < nq:
                g = Q * QUAD // G
                if (Q * QUAD) % G == 0:
                    gcol = g * G * T
                    xlo = px.tile([128, G * T], BF16, tag="xlo")
                    xhi = px.tile([64, G * T], BF16, tag="xhi")
                    nc.sync.dma_start(out=xlo, in_=xt[0:128, gcol : gcol + G * T])
                    nc.sync.dma_start(out=xhi, in_=xt[128:192, gcol : gcol + G * T])
                    x_tiles[g] = (xlo, xhi)
                xlo, xhi = x_tiles[g]
                qs = (Q * QUAD * T) % (G * T)
                qcols = slice(qs, qs + QUAD * T)

                # gT = A.T @ xT for 4 batches; lo rows in bank0, hi rows
                # (64 partitions) in bank1 of one 2-bank tile.
                gt = pg.tile([128, 1024], F32, tag="gt")
                nc.tensor.matmul(gt[:, 0:512], a_lo[:, 0:128], xlo[:, qcols],
                                 start=True, stop=False)
                nc.tensor.matmul(gt[:, 0:512], a_hi[:, 0:128], xhi[:, qcols],
                                 start=False, stop=True)
                nc.tensor.matmul(gt[0:64, 512:1024], a_lo[:, 128:192],
                                 xlo[:, qcols], start=True, stop=False)
                nc.tensor.matmul(gt[0:64, 512:1024], a_hi[:, 128:192],
                                 xhi[:, qcols], start=False, stop=True)
                gsb = pgsb.tile([128, 1024], BF16, tag="gsb")
                nc.scalar.copy(out=gsb[:, 0:512], in_=gt[:, 0:512])
                nc.vector.tensor_copy(out=gsb[0:64, 512:1024],
                                      in_=gt[0:64, 512:1024])
                gsb_t[Q] = gsb

                # v = xT.T @ WvT, two batches per PSUM bank; ones column for
                # the softmax row sums rides at col 192 of v_sb.
                for pr in range(QUAD // 2):
                    v_ps = pv.tile([128, 2, 256], F32, tag="v_ps")
                    for jj in range(2):
                        bs = qs + (pr * 2 + jj) * T
                        nc.tensor.matmul(v_ps[:, jj, 0:H], xlo[:, bs : bs + T],
                                         wvt_lo, start=True, stop=False)
                        nc.tensor.matmul(v_ps[:, jj, 0:H], xhi[:, bs : bs + T],
                                         wvt_hi, start=False, stop=True)
                    v_sb = pvsb.tile([128, 2, 200], BF16, tag="v_sb")
                    nc.gpsimd.memset(v_sb[:, :, H : H + 1], 1.0)
                    nc.vector.tensor_copy(out=v_sb[:, :, 0:H],
                                          in_=v_ps[:, :, 0:H])
                    vsb_t[(Q, pr)] = v_sb

            # weiT(P) = xT.T @ gT as two independent single matmuls per
            # batch (K=128 into bank A, K=64 into bank B), summed on DVE.
            if 1 <= Q <= nq:
                P = Q - 1
                pg_ = P * QUAD // G
                xlo_p, xhi_p = x_tiles[pg_]
                ps_ = (P * QUAD * T) % (G * T)
                gsb = gsb_t.pop(P)
                wa = pwa.tile([128, QUAD, T], F32, tag="wa")
                wb = pwb.tile([128, QUAD, T], F32, tag="wb")
                for j in range(QUAD):
                    ks = ps_ + j * T
                    jc = slice(j * T, (j + 1) * T)
                    nc.tensor.matmul(wa[:, j, :], xlo_p[:, ks : ks + T],
                                     gsb[:, jc], start=True, stop=True)
                    nc.tensor.matmul(wb[:, j, :], xhi_p[:, ks : ks + T],
                                     gsb[0:64, 512 + j * T : 512 + (j + 1) * T],
                                     start=True, stop=True)
                # exp(wa + wb) = exp(wa) * exp(wb): both exps read PSUM on
                # ScalarE; the causal mask folds onto the first factor on
                # GpSimd; DVE combines at 2x bf16 rate.
                pA = ppa.tile([128, QUAD, T], BF16, tag="pA")
                nc.scalar.activation(out=pA, in_=wa, func=Exp)
                pB = ppb.tile([128, QUAD, T], BF16, tag="pB")
                nc.scalar.activation(out=pB, in_=wb, func=Exp)
                # causal mask: keep where q >= k  (k = partition index)
                pAm = ppa.tile([128, QUAD, T], BF16, tag="pAm")
                nc.gpsimd.affine_select(
                    out=pAm, in_=pA,
                    compare_op=mybir.AluOpType.is_ge,
                    fill=0.0, base=0, channel_multiplier=-1,
                    pattern=[[0, QUAD], [1, T]],
                )
                pmT = ppm.tile([128, QUAD, T], BF16, tag="pmT")
                nc.vector.tensor_mul(pmT, pAm, pB)
                pm_t[P] = pmT
                # Free the x group once its last quad's weiT is emitted.
                if (P + 1) * QUAD % G == 0 and (P + 1) * QUAD // G > 0:
                    pass

            # o(O) = PmT.T @ v_ext ; col H = softmax denominator
            if Q >= 2:
                O_ = Q - 2
                pmT = pm_t.pop(O_)
                gb = O_ * QUAD // G
                ob0 = (O_ * QUAD) % G
                if ob0 == 0:
                    osb_t[gb] = posb.tile([128, G, H], F32, tag="o_sb",
                                          name="o_sb")
                o_sb = osb_t[gb]
                for pr in range(QUAD // 2):
                    v_sb = vsb_t.pop((O_, pr))
                    o_ps = po.tile([128, 2, 256], F32, tag="o_ps")
                    for jj in range(2):
                        j = pr * 2 + jj
                        nc.tensor.matmul(o_ps[:, jj, 0 : H + 1], pmT[:, j, :],
                                         v_sb[:, jj, 0 : H + 1],
                                         start=True, stop=True)
                    r = psr.tile([128, 2], F32, tag="r")
                    nc.vector.reciprocal(out=r, in_=o_ps[:, :, H])
                    ob = ob0 + pr * 2
                    nc.vector.tensor_scalar_mul(
                        out=o_sb[:, ob, :], in0=o_ps[:, 0, 0:H],
                        scalar1=r[:, 0:1],
                    )
                    nc.scalar.mul(out=o_sb[:, ob + 1, :], in_=o_ps[:, 1, 0:H],
                                  mul=r[:, 1:2])
                if ob0 + QUAD == G:
                    nc.sync.dma_start(
                        out=o[:, gb * G : (gb + 1) * G, :], in_=o_sb
                    )
                    del osb_t[gb]
    return nc


_cached = {}


def _get_nc(nb):
    if nb not in _cached:
        _cached[nb] = build_nc(nb)
    return _cached[nb]


def prep_inputs(x, Wq, Wk, Wv, nb=NB, ncores=NCORES):
    """Host-side sharding + layout/dtype prep + weight folding."""
    x = np.asarray(x, dtype=np.float32)
    A = (np.asarray(Wq, np.float32).T @ np.asarray(Wk, np.float32)) * SCALE
    a_bf = np.ascontiguousarray(A).astype(NPBF16)
    wvt_bf = np.ascontiguousarray(np.asarray(Wv, np.float32).T).astype(NPBF16)
    in_maps = []
    for c in range(ncores):
        shard = x[c * nb : (c + 1) * nb]                      # [nb, T, E]
        xt = np.ascontiguousarray(shard.transpose(2, 0, 1)).reshape(E, nb * T)
        in_maps.append({"xt": xt.astype(NPBF16), "a": a_bf, "wvt": wvt_bf})
    return in_maps


def kernel(x, Wq, Wk, Wv, _trace=False):
    nc = _get_nc(NB)
    in_maps = prep_inputs(x, Wq, Wk, Wv)
    res = run_bass_kernel_spmd(
        nc, in_maps, core_ids=list(range(NCORES)), trace=_trace
    )
    # o is [T, nb, H] per core; transpose to [nb, T, H] and concat.
    out = np.concatenate(
        [res.results[c]["o"].transpose(1, 0, 2) for c in range(NCORES)], axis=0
    )
    out = np.ascontiguousarray(out, dtype=np.float32)
    if _trace:
        kernel.last_result = res
    return out


# revision 6
# speedup vs baseline: 1.7229x; 1.4439x over previous
"""Trainium2 Bass kernel for nn_Head (single-head causal self-attention).

Module:  q = x@Wq.T, k = x@Wk.T, v = x@Wv.T
         wei = softmax(causal_mask(q@k.T * E**-0.5))
         out = wei @ v
Shapes:  x [2048, 128, 192], Wq/Wk/Wv [192, 192] -> out [2048, 128, 192]

Strategy (pure data parallel over the batch dim, 8 cores x 256 batches):
  - Weight fold: wei = x @ A @ x.T with A = (Wq.T @ Wk) * SCALE, so only one
    projection ("g = x @ A") is needed for the attention logits.
  - Host pads the feature dim E 192 -> 256 with zeros and ships x transposed
    per-core as xt[e, b*T + t] in bf16.  The pad makes every K-chained
    matmul a full K=128 x K=128 pair: half-empty K=64 chain matmuls keep
    the PE's HAM activity monitor below its un-throttle threshold and lock
    the whole kernel at 1.2 GHz.
  - Logits are computed TRANSPOSED (weiT[k, q]) so the masked exp'd tile is
    directly the lhsT of the output matmul -- no PE transposes.
  - weiT's two K-halves are two independent single matmuls into separate
    PSUM banks; exp(wa+wb) = exp(wa)*exp(wb) merges them (ScalarE exp x2 +
    GpSimd mask + DVE multiply) -- single matmuls hide their LDWEIGHTS.
  - g and v keep PSUM accumulation chains but interleave the two chains of
    each quad across banks so chain LDWEIGHTS loads hide under the other
    chain's streaming.
  - Row sums ride along the output matmul via a ones-column on v; ScalarE /
    DVE normalize with per-partition reciprocals.
"""

import os
import sys

sys.path.insert(0, "/opt/trn_rl_repo")

import numpy as np
import ml_dtypes
from contextlib import ExitStack

import json

import concourse.bass as bass
import concourse.bass2jax as bass2jax
import concourse.mybir as mybir
import concourse.tile as tile
from concourse.bass_utils import (
    compile_bir_kernel as _orig_compile_bir_kernel,
    run_bass_kernel_spmd,
)

BF16 = mybir.dt.bfloat16
F32 = mybir.dt.float32
NPBF16 = ml_dtypes.bfloat16

B, T, E, H = 2048, 128, 192, 192
EP = 256                    # zero-padded feature dim (2 x 128)
NCORES = 8
NB = B // NCORES            # batches per core
SCALE = float(E) ** -0.5
G = 8                       # batches per DMA group
QUAD = 4                    # batches per pipeline stage
NGROUPS = NB // G


def _patch_tile_tail_drain():
    """Walrus rejects the TileContext tail Drain when it carries more than a
    couple of sem waits ("Too many sync wait commands").  Redistribute the
    waits onto single-wait SP nops emitted between the drain and barrier."""
    if getattr(tile.TileContext, "_tail_drain_patched", False):
        return

    def _drain_and_barrier(self, tick_clock, wait_clock):
        from concourse.tile import ScopedClock

        drain_inst = self.nc.sync.drain()
        wait_clock.add_sem_waits(
            drain_inst.ins, ScopedClock({None: tick_clock.global_clock})
        )
        waits = list(drain_inst.ins.sync_info.on_wait or [])
        if len(waits) > 1:
            drain_inst.ins.sync_info = mybir.SyncInfo(
                on_wait=[waits[0]], on_update=[]
            )
            for w in waits[1:]:
                nop = self.nc.sync.nop()
                nop.ins.sync_info = mybir.SyncInfo(on_wait=[w], on_update=[])
        self.nc.all_engine_barrier()
        assert self.sems is not None
        popped = self.nc._tile_sem_poison_stack.pop()
        assert popped is self._sem_poison
        self.nc.clear_and_free_semaphores(list(self.sems.allocated().values()))
        self.nc.all_engine_barrier()

    tile.TileContext._drain_and_barrier = _drain_and_barrier
    tile.TileContext._tail_drain_patched = True


def _split_multi_waits(bir_json: bytes) -> bytes:
    """This container's walrus supports only ONE sync-wait slot per
    instruction ("Too many sync wait commands").  Hoist extra waits onto
    single-wait NoOps inserted just before the instruction (same engine, so
    per-engine program order and blocking semantics are preserved)."""
    d = json.loads(bir_json)
    n = 0
    for f in d.get("functions", []):
        for bb in f.get("blocks", []):
            insts = bb.get("instructions", [])
            out = []
            changed = False
            for inst in insts:
                si = inst.get("sync_info")
                waits = (si.get("on_wait") or []) if si else []
                if len(waits) > 1:
                    changed = True
                    for w in waits[:-1]:
                        n += 1
                        out.append({
                            "debug": inst.get("debug"),
                            "engine": inst["engine"],
                            "ins": [],
                            "name": f"WSPLIT-{n}",
                            "opcode": "NoOp",
                            "outs": [],
                            "sync_info": {"on_update": [], "on_wait": [w]},
                        })
                    si["on_wait"] = [waits[-1]]
                out.append(inst)
            if changed:
                bb["instructions"] = out
    if n == 0:
        return bir_json
    return json.dumps(d).encode()


def _patched_compile_bir_kernel(bir_json, tmpdir, neff_name="file.neff"):
    if isinstance(bir_json, str):
        bir_json = bir_json.encode()
    return _orig_compile_bir_kernel(_split_multi_waits(bir_json), tmpdir, neff_name)


bass2jax.compile_bir_kernel = _patched_compile_bir_kernel


def build_nc(nb=NB):
    _patch_tile_tail_drain()
    nc = bass.Bass(trn_type="TRN2")

    xt = nc.dram_tensor("xt", [EP, nb * T], BF16, kind="ExternalInput")
    a = nc.dram_tensor("a", [EP, E], BF16, kind="ExternalInput")
    wvt = nc.dram_tensor("wvt", [EP, H], BF16, kind="ExternalInput")
    # Output laid out [t, b, h] for contiguous DMA; host transposes.
    o = nc.dram_tensor("o", [T, nb, H], F32, kind="ExternalOutput")

    nq = nb // QUAD
    Exp = mybir.ActivationFunctionType.Exp

    with tile.TileContext(nc) as tc, ExitStack() as ctx:
        singles = ctx.enter_context(tc.tile_pool(name="singles", bufs=1))
        px = ctx.enter_context(tc.tile_pool(name="px", bufs=4))
        pgsb = ctx.enter_context(tc.tile_pool(name="pgsb", bufs=3))
        ppa = ctx.enter_context(tc.tile_pool(name="ppa", bufs=2))
        ppb = ctx.enter_context(tc.tile_pool(name="ppb", bufs=2))
        ppm = ctx.enter_context(tc.tile_pool(name="ppm", bufs=3))
        pvsb = ctx.enter_context(tc.tile_pool(name="pvsb", bufs=6))
        psr = ctx.enter_context(tc.tile_pool(name="psr", bufs=4))
        posb = ctx.enter_context(tc.tile_pool(name="posb", bufs=2))

        pg = ctx.enter_context(tc.tile_pool(name="pg", bufs=1, space="PSUM"))
        pwa = ctx.enter_context(tc.tile_pool(name="pwa", bufs=1, space="PSUM"))
        pwb = ctx.enter_context(tc.tile_pool(name="pwb", bufs=1, space="PSUM"))
        pv = ctx.enter_context(tc.tile_pool(name="pv", bufs=2, space="PSUM"))
        po = ctx.enter_context(tc.tile_pool(name="po", bufs=2, space="PSUM"))

        # Constants: A (lhsT for gT), WvT (rhs for v); hi halves zero-padded
        # to full 128 partitions.
        a_lo = singles.tile([128, E], BF16, tag="a_lo")
        a_hi = singles.tile([128, E], BF16, tag="a_hi")
        nc.sync.dma_start(out=a_lo, in_=a[0:128, :])
        nc.sync.dma_start(out=a_hi, in_=a[128:256, :])
        wvt_lo = singles.tile([128, H], BF16, tag="wvt_lo")
        wvt_hi = singles.tile([128, H], BF16, tag="wvt_hi")
        nc.sync.dma_start(out=wvt_lo, in_=wvt[0:128, :])
        nc.sync.dma_start(out=wvt_hi, in_=wvt[128:256, :])

        # Software pipeline over quads: iteration Q emits
        #   gT(Q), v(Q)  ->  weiT(Q-1) + exp + mask  ->  o(Q-2)
        x_tiles = {}     # group -> (xlo, xhi)
        gsb_t = {}       # Q -> gsb ([128,1024]: gT_lo | gT_hi)
        pm_t = {}        # Q -> masked exp'd weiT (PmT)
        vsb_t = {}       # (Q, pr) -> v_sb pair
        osb_t = {}       # group -> o_sb

        for Q in range(nq + 2):
            if Q < nq:
                g = Q * QUAD // G
                if (Q * QUAD) % G == 0:
                    gcol = g * G * T
                    xlo = px.tile([128, G * T], BF16, tag="xlo")
                    xhi = px.tile([128, G * T], BF16, tag="xhi")
                    nc.sync.dma_start(out=xlo, in_=xt[0:128, gcol : gcol + G * T])
                    nc.sync.dma_start(out=xhi, in_=xt[128:256, gcol : gcol + G * T])
                    x_tiles[g] = (xlo, xhi)
                xlo, xhi = x_tiles[g]
                qs = (Q * QUAD * T) % (G * T)
                qcols = slice(qs, qs + QUAD * T)

                # gT = A.T @ xT for 4 batches; the lo-rows chain (bank0) and
                # hi-rows chain (bank1) are interleaved so each chain's
                # LDWEIGHTS hides under the other chain's streaming.
                gt = pg.tile([128, 1024], F32, tag="gt")
                nc.tensor.matmul(gt[:, 0:512], a_lo[:, 0:128], xlo[:, qcols],
                                 start=True, stop=False)
                nc.tensor.matmul(gt[0:64, 512:1024], a_lo[:, 128:192],
                                 xlo[:, qcols], start=True, stop=False)
                nc.tensor.matmul(gt[:, 0:512], a_hi[:, 0:128], xhi[:, qcols],
                                 start=False, stop=True)
                nc.tensor.matmul(gt[0:64, 512:1024], a_hi[:, 128:192],
                                 xhi[:, qcols], start=False, stop=True)
                gsb = pgsb.tile([128, 1024], BF16, tag="gsb")
                nc.scalar.copy(out=gsb[:, 0:512], in_=gt[:, 0:512])
                nc.vector.tensor_copy(out=gsb[0:64, 512:1024],
                                      in_=gt[0:64, 512:1024])
                gsb_t[Q] = gsb

                # v = xT.T @ WvT, two batches per PSUM bank, two banks (pair
                # tiles) with their chains interleaved across banks.
                v_ps0 = pv.tile([128, 2, 256], F32, tag="v_ps", name="v_ps0")
                v_ps1 = pv.tile([128, 2, 256], F32, tag="v_ps", name="v_ps1")
                for jj in range(2):
                    b0 = qs + jj * T
                    b1 = qs + (2 + jj) * T
                    nc.tensor.matmul(v_ps0[:, jj, 0:H], xlo[:, b0 : b0 + T],
                                     wvt_lo, start=True, stop=False)
                    nc.tensor.matmul(v_ps1[:, jj, 0:H], xlo[:, b1 : b1 + T],
                                     wvt_lo, start=True, stop=False)
                    nc.tensor.matmul(v_ps0[:, jj, 0:H], xhi[:, b0 : b0 + T],
                                     wvt_hi, start=False, stop=True)
                    nc.tensor.matmul(v_ps1[:, jj, 0:H], xhi[:, b1 : b1 + T],
                                     wvt_hi, start=False, stop=True)
                for pr, v_ps in ((0, v_ps0), (1, v_ps1)):
                    v_sb = pvsb.tile([128, 2, 200], BF16, tag="v_sb")
                    nc.gpsimd.memset(v_sb[:, :, H : H + 1], 1.0)
                    nc.vector.tensor_copy(out=v_sb[:, :, 0:H],
                                          in_=v_ps[:, :, 0:H])
                    vsb_t[(Q, pr)] = v_sb

            # weiT(P) = xT.T @ gT as two independent single matmuls per
            # batch (K=128 into bank A, K=64 into bank B); exp factorizes.
            if 1 <= Q <= nq:
                P = Q - 1
                pg_ = P * QUAD // G
                xlo_p, xhi_p = x_tiles[pg_]
                ps_ = (P * QUAD * T) % (G * T)
                gsb = gsb_t.pop(P)
                wa = pwa.tile([128, QUAD, T], F32, tag="wa")
                wb = pwb.tile([128, QUAD, T], F32, tag="wb")
                for j in range(QUAD):
                    ks = ps_ + j * T
                    jc = slice(j * T, (j + 1) * T)
                    nc.tensor.matmul(wa[:, j, :], xlo_p[:, ks : ks + T],
                                     gsb[:, jc], start=True, stop=True)
                    nc.tensor.matmul(wb[:, j, :], xhi_p[0:64, ks : ks + T],
                                     gsb[0:64, 512 + j * T : 512 + (j + 1) * T],
                                     start=True, stop=True)
                # exp(wa + wb) = exp(wa) * exp(wb); mask folds onto the
                # first factor on GpSimd; GpSimd also combines at bf16.
                pA = ppa.tile([128, QUAD, T], BF16, tag="pA")
                nc.scalar.activation(out=pA, in_=wa, func=Exp)
                pB = ppb.tile([128, QUAD, T], BF16, tag="pB")
                nc.scalar.activation(out=pB, in_=wb, func=Exp)
                # causal mask: keep where q >= k  (k = partition index)
                pAm = ppa.tile([128, QUAD, T], BF16, tag="pAm")
                nc.gpsimd.affine_select(
                    out=pAm, in_=pA,
                    compare_op=mybir.AluOpType.is_ge,
                    fill=0.0, base=0, channel_multiplier=-1,
                    pattern=[[0, QUAD], [1, T]],
                )
                pmT = ppm.tile([128, QUAD, T], BF16, tag="pmT")
                nc.gpsimd.tensor_mul(pmT, pAm, pB)
                pm_t[P] = pmT

            # o(O) = PmT.T @ v_ext ; col H = softmax denominator
            if Q >= 2:
                O_ = Q - 2
                pmT = pm_t.pop(O_)
                gb = O_ * QUAD // G
                ob0 = (O_ * QUAD) % G
                if ob0 == 0:
                    osb_t[gb] = posb.tile([128, G, H], F32, tag="o_sb",
                                          name="o_sb")
                o_sb = osb_t[gb]
                for pr in range(QUAD // 2):
                    v_sb = vsb_t.pop((O_, pr))
                    o_ps = po.tile([128, 2, 256], F32, tag="o_ps")
                    for jj in range(2):
                        j = pr * 2 + jj
                        nc.tensor.matmul(o_ps[:, jj, 0 : H + 1], pmT[:, j, :],
                                         v_sb[:, jj, 0 : H + 1],
                                         start=True, stop=True)
                    r = psr.tile([128, 2], F32, tag="r")
                    nc.vector.reciprocal(out=r, in_=o_ps[:, :, H])
                    ob = ob0 + pr * 2
                    nc.vector.tensor_scalar_mul(
                        out=o_sb[:, ob, :], in0=o_ps[:, 0, 0:H],
                        scalar1=r[:, 0:1],
                    )
                    nc.scalar.mul(out=o_sb[:, ob + 1, :], in_=o_ps[:, 1, 0:H],
                                  mul=r[:, 1:2])
                if ob0 + QUAD == G:
                    nc.sync.dma_start(
                        out=o[:, gb * G : (gb + 1) * G, :], in_=o_sb
                    )
                    del osb_t[gb]
    return nc


_cached = {}


def _get_nc(nb):
    if nb not in _cached:
        _cached[nb] = build_nc(nb)
    return _cached[nb]


def prep_inputs(x, Wq, Wk, Wv, nb=NB, ncores=NCORES):
    """Host-side sharding + layout/dtype prep + weight folding + zero-pad."""
    x = np.asarray(x, dtype=np.float32)
    A = (np.asarray(Wq, np.float32).T @ np.asarray(Wk, np.float32)) * SCALE
    a_bf = np.zeros((EP, E), dtype=NPBF16)
    a_bf[0:E] = A.astype(NPBF16)
    wvt_bf = np.zeros((EP, H), dtype=NPBF16)
    wvt_bf[0:E] = np.ascontiguousarray(np.asarray(Wv, np.float32).T).astype(NPBF16)
    in_maps = []
    for c in range(ncores):
        shard = x[c * nb : (c + 1) * nb]                      # [nb, T, E]
        xt = np.zeros((EP, nb * T), dtype=NPBF16)
        xt[0:E] = (
            np.ascontiguousarray(shard.transpose(2, 0, 1))
            .reshape(E, nb * T)
            .astype(NPBF16)
        )
        in_maps.append({"xt": xt, "a": a_bf, "wvt": wvt_bf})
    return in_maps


def kernel(x, Wq, Wk, Wv, _trace=False):
    nc = _get_nc(NB)
    in_maps = prep_inputs(x, Wq, Wk, Wv)
    res = run_bass_kernel_spmd(
        nc, in_maps, core_ids=list(range(NCORES)), trace=_trace
    )
    # o is [T, nb, H] per core; transpose to [nb, T, H] and concat.
    out = np.concatenate(
        [res.results[c]["o"].transpose(1, 0, 2) for c in range(NCORES)], axis=0
    )
    out = np.ascontiguousarray(out, dtype=np.float32)
    if _trace:
        kernel.last_result = res
    return out


# revision 8
# speedup vs baseline: 2.0023x; 1.1621x over previous
"""Trainium2 Bass kernel for nn_Head (single-head causal self-attention).

Module:  q = x@Wq.T, k = x@Wk.T, v = x@Wv.T
         wei = softmax(causal_mask(q@k.T * E**-0.5))
         out = wei @ v
Shapes:  x [2048, 128, 192], Wq/Wk/Wv [192, 192] -> out [2048, 128, 192]

Strategy (pure data parallel over the batch dim, 8 cores x 256 batches):
  - Weight fold: wei = x @ A @ x.T with A = (Wq.T @ Wk) * SCALE, so only one
    projection ("g = x @ A") is needed for the attention logits.
  - Host pads the feature dim E 192 -> 256 with zeros and ships x transposed
    per-core as xt[e, b*T + t] in bf16.  The pad makes every K-chained
    matmul a full K=128 x K=128 pair: half-empty K=64 chain matmuls keep
    the PE's HAM activity monitor below its un-throttle threshold and lock
    the whole kernel at 1.2 GHz.
  - Logits are computed TRANSPOSED (weiT[k, q]) so the masked exp'd tile is
    directly the lhsT of the output matmul -- no PE transposes.
  - weiT's two K-halves are two independent single matmuls into separate
    PSUM banks; exp(wa+wb) = exp(wa)*exp(wb) merges them (ScalarE exp x2 +
    GpSimd mask + DVE multiply) -- single matmuls hide their LDWEIGHTS.
  - g and v keep PSUM accumulation chains but interleave the two chains of
    each quad across banks so chain LDWEIGHTS loads hide under the other
    chain's streaming.
  - Row sums ride along the output matmul via a ones-column on v; ScalarE /
    DVE normalize with per-partition reciprocals.
"""

import os
import sys

sys.path.insert(0, "/opt/trn_rl_repo")

import numpy as np
import ml_dtypes
from contextlib import ExitStack

import json

import concourse.bass as bass
import concourse.bass2jax as bass2jax
import concourse.mybir as mybir
import concourse.tile as tile
from concourse.bass_utils import (
    compile_bir_kernel as _orig_compile_bir_kernel,
    run_bass_kernel_spmd,
)

BF16 = mybir.dt.bfloat16
F32 = mybir.dt.float32
NPBF16 = ml_dtypes.bfloat16

B, T, E, H = 2048, 128, 192, 192
EP = 256                    # zero-padded feature dim (2 x 128)
NCORES = 8
NB = B // NCORES            # batches per core
SCALE = float(E) ** -0.5
G = 8                       # batches per DMA group
QUAD = 4                    # batches per pipeline stage
NGROUPS = NB // G


def _patch_tile_tail_drain():
    """Walrus rejects the TileContext tail Drain when it carries more than a
    couple of sem waits ("Too many sync wait commands").  Redistribute the
    waits onto single-wait SP nops emitted between the drain and barrier."""
    if getattr(tile.TileContext, "_tail_drain_patched", False):
        return

    def _drain_and_barrier(self, tick_clock, wait_clock):
        from concourse.tile import ScopedClock

        drain_inst = self.nc.sync.drain()
        wait_clock.add_sem_waits(
            drain_inst.ins, ScopedClock({None: tick_clock.global_clock})
        )
        waits = list(drain_inst.ins.sync_info.on_wait or [])
        if len(waits) > 1:
            drain_inst.ins.sync_info = mybir.SyncInfo(
                on_wait=[waits[0]], on_update=[]
            )
            for w in waits[1:]:
                nop = self.nc.sync.nop()
                nop.ins.sync_info = mybir.SyncInfo(on_wait=[w], on_update=[])
        self.nc.all_engine_barrier()
        assert self.sems is not None
        popped = self.nc._tile_sem_poison_stack.pop()
        assert popped is self._sem_poison
        self.nc.clear_and_free_semaphores(list(self.sems.allocated().values()))
        self.nc.all_engine_barrier()

    tile.TileContext._drain_and_barrier = _drain_and_barrier
    tile.TileContext._tail_drain_patched = True


def _split_multi_waits(bir_json: bytes) -> bytes:
    """This container's walrus supports only ONE sync-wait slot per
    instruction ("Too many sync wait commands").  Hoist extra waits onto
    single-wait NoOps inserted just before the instruction (same engine, so
    per-engine program order and blocking semantics are preserved)."""
    d = json.loads(bir_json)
    n = 0
    for f in d.get("functions", []):
        for bb in f.get("blocks", []):
            insts = bb.get("instructions", [])
            out = []
            changed = False
            for inst in insts:
                si = inst.get("sync_info")
                waits = (si.get("on_wait") or []) if si else []
                if len(waits) > 1:
                    changed = True
                    for w in waits[:-1]:
                        n += 1
                        out.append({
                            "debug": inst.get("debug"),
                            "engine": inst["engine"],
                            "ins": [],
                            "name": f"WSPLIT-{n}",
                            "opcode": "NoOp",
                            "outs": [],
                            "sync_info": {"on_update": [], "on_wait": [w]},
                        })
                    si["on_wait"] = [waits[-1]]
                out.append(inst)
            if changed:
                bb["instructions"] = out
    if n == 0:
        return bir_json
    return json.dumps(d).encode()


def _patched_compile_bir_kernel(bir_json, tmpdir, neff_name="file.neff"):
    if isinstance(bir_json, str):
        bir_json = bir_json.encode()
    return _orig_compile_bir_kernel(_split_multi_waits(bir_json), tmpdir, neff_name)


bass2jax.compile_bir_kernel = _patched_compile_bir_kernel


def build_nc(nb=NB):
    _patch_tile_tail_drain()
    nc = bass.Bass(trn_type="TRN2")

    xt = nc.dram_tensor("xt", [EP, nb * T], BF16, kind="ExternalInput")
    a = nc.dram_tensor("a", [EP, E], BF16, kind="ExternalInput")
    wvt = nc.dram_tensor("wvt", [EP, H], BF16, kind="ExternalInput")
    # Output laid out [t, b, h] for contiguous DMA; host transposes.
    o = nc.dram_tensor("o", [T, nb, H], F32, kind="ExternalOutput")

    nq = nb // QUAD
    Exp = mybir.ActivationFunctionType.Exp

    with tile.TileContext(nc) as tc, ExitStack() as ctx:
        singles = ctx.enter_context(tc.tile_pool(name="singles", bufs=1))
        px = ctx.enter_context(tc.tile_pool(name="px", bufs=4))
        pgsb = ctx.enter_context(tc.tile_pool(name="pgsb", bufs=3))
        ppa = ctx.enter_context(tc.tile_pool(name="ppa", bufs=2))
        ppb = ctx.enter_context(tc.tile_pool(name="ppb", bufs=2))
        ppm = ctx.enter_context(tc.tile_pool(name="ppm", bufs=3))
        pvsb = ctx.enter_context(tc.tile_pool(name="pvsb", bufs=6))
        psr = ctx.enter_context(tc.tile_pool(name="psr", bufs=4))
        posb = ctx.enter_context(tc.tile_pool(name="posb", bufs=2))

        pg = ctx.enter_context(tc.tile_pool(name="pg", bufs=1, space="PSUM"))
        pwa = ctx.enter_context(tc.tile_pool(name="pwa", bufs=1, space="PSUM"))
        pwb = ctx.enter_context(tc.tile_pool(name="pwb", bufs=1, space="PSUM"))
        pv = ctx.enter_context(tc.tile_pool(name="pv", bufs=2, space="PSUM"))
        po = ctx.enter_context(tc.tile_pool(name="po", bufs=2, space="PSUM"))

        # Constants: A (lhsT for gT), WvT (rhs for v); hi halves zero-padded
        # to full 128 partitions.
        a_lo = singles.tile([128, E], BF16, tag="a_lo")
        a_hi = singles.tile([128, E], BF16, tag="a_hi")
        nc.sync.dma_start(out=a_lo, in_=a[0:128, :])
        nc.sync.dma_start(out=a_hi, in_=a[128:256, :])
        wvt_lo = singles.tile([128, H], BF16, tag="wvt_lo")
        wvt_hi = singles.tile([128, H], BF16, tag="wvt_hi")
        nc.sync.dma_start(out=wvt_lo, in_=wvt[0:128, :])
        nc.sync.dma_start(out=wvt_hi, in_=wvt[128:256, :])

        # Software pipeline over quads: iteration Q emits
        #   gT(Q), v(Q)  ->  weiT(Q-1) + exp + mask  ->  o(Q-2)
        x_tiles = {}     # group -> (xlo, xhi)
        gsb_t = {}       # Q -> gsb ([128,1024]: gT_lo | gT_hi)
        pm_t = {}        # Q -> masked exp'd weiT (PmT)
        vsb_t = {}       # (Q, pr) -> v_sb pair
        osb_t = {}       # group -> o_sb

        for Q in range(nq + 2):
            if Q < nq:
                g = Q * QUAD // G
                if (Q * QUAD) % G == 0:
                    gcol = g * G * T
                    xlo = px.tile([128, G * T], BF16, tag="xlo")
                    xhi = px.tile([128, G * T], BF16, tag="xhi")
                    nc.sync.dma_start(out=xlo, in_=xt[0:128, gcol : gcol + G * T])
                    nc.sync.dma_start(out=xhi, in_=xt[128:256, gcol : gcol + G * T])
                    x_tiles[g] = (xlo, xhi)
                xlo, xhi = x_tiles[g]
                qs = (Q * QUAD * T) % (G * T)
                qcols = slice(qs, qs + QUAD * T)

                # gT = A.T @ xT for 4 batches; the lo-rows chain (bank0) and
                # hi-rows chain (bank1) are interleaved so each chain's
                # LDWEIGHTS hides under the other chain's streaming.
                gt = pg.tile([128, 1024], F32, tag="gt")
                nc.tensor.matmul(gt[:, 0:512], a_lo[:, 0:128], xlo[:, qcols],
                                 start=True, stop=False)
                nc.tensor.matmul(gt[0:64, 512:1024], a_lo[:, 128:192],
                                 xlo[:, qcols], start=True, stop=False)
                nc.tensor.matmul(gt[:, 0:512], a_hi[:, 0:128], xhi[:, qcols],
                                 start=False, stop=True)
                nc.tensor.matmul(gt[0:64, 512:1024], a_hi[:, 128:192],
                                 xhi[:, qcols], start=False, stop=True)
                gsb = pgsb.tile([128, 1024], BF16, tag="gsb")
                nc.scalar.copy(out=gsb[:, 0:512], in_=gt[:, 0:512])
                nc.vector.tensor_copy(out=gsb[0:64, 512:1024],
                                      in_=gt[0:64, 512:1024])
                gsb_t[Q] = gsb

                # v = xT.T @ WvT, two batches per PSUM bank, two banks (pair
                # tiles) with their chains interleaved across banks.
                v_ps0 = pv.tile([128, 2, 256], F32, tag="v_ps", name="v_ps0")
                v_ps1 = pv.tile([128, 2, 256], F32, tag="v_ps", name="v_ps1")
                for jj in range(2):
                    b0 = qs + jj * T
                    b1 = qs + (2 + jj) * T
                    nc.tensor.matmul(v_ps0[:, jj, 0:H], xlo[:, b0 : b0 + T],
                                     wvt_lo, start=True, stop=False)
                    nc.tensor.matmul(v_ps1[:, jj, 0:H], xlo[:, b1 : b1 + T],
                                     wvt_lo, start=True, stop=False)
                    nc.tensor.matmul(v_ps0[:, jj, 0:H], xhi[:, b0 : b0 + T],
                                     wvt_hi, start=False, stop=True)
                    nc.tensor.matmul(v_ps1[:, jj, 0:H], xhi[:, b1 : b1 + T],
                                     wvt_hi, start=False, stop=True)
                for pr, v_ps in ((0, v_ps0), (1, v_ps1)):
                    v_sb = pvsb.tile([128, 2, 200], BF16, tag="v_sb")
                    nc.gpsimd.memset(v_sb[:, :, H : H + 1], 1.0)
                    nc.vector.tensor_copy(out=v_sb[:, :, 0:H],
                                          in_=v_ps[:, :, 0:H])
                    vsb_t[(Q, pr)] = v_sb

            # weiT(P) = xT.T @ gT as full-K (padded) chains: batch pair 0
            # in bank A, pair 1 in bank B, chains interleaved across banks
            # so each chain's LDWEIGHTS hides under the other's streaming.
            if 1 <= Q <= nq:
                P = Q - 1
                pg_ = P * QUAD // G
                xlo_p, xhi_p = x_tiles[pg_]
                ps_ = (P * QUAD * T) % (G * T)
                gsb = gsb_t.pop(P)
                if P < 3:
                    # The padded wei chains read gsb rows 64:128 of the hi
                    # half (zero lhsT rows x junk = NaN risk); zero each
                    # pool buffer once.
                    nc.gpsimd.memset(gsb[64:128, 512:1024], 0.0)
                wa = pwa.tile([128, 2, T], F32, tag="wa")
                wb = pwb.tile([128, 2, T], F32, tag="wb")
                for jj in range(2):
                    ka = ps_ + jj * T
                    kb = ps_ + (2 + jj) * T
                    ja = slice(jj * T, (jj + 1) * T)
                    jb = slice((2 + jj) * T, (3 + jj) * T)
                    nc.tensor.matmul(wa[:, jj, :], xlo_p[:, ka : ka + T],
                                     gsb[:, ja], start=True, stop=False)
                    nc.tensor.matmul(wb[:, jj, :], xlo_p[:, kb : kb + T],
                                     gsb[:, jb], start=True, stop=False)
                    nc.tensor.matmul(wa[:, jj, :], xhi_p[:, ka : ka + T],
                                     gsb[:, 512 + jj * T : 512 + (jj + 1) * T],
                                     start=False, stop=True)
                    nc.tensor.matmul(wb[:, jj, :], xhi_p[:, kb : kb + T],
                                     gsb[:, 512 + (2 + jj) * T : 512 + (3 + jj) * T],
                                     start=False, stop=True)
                pA = ppa.tile([128, 2, T], BF16, tag="pA")
                nc.scalar.activation(out=pA, in_=wa, func=Exp)
                pB = ppb.tile([128, 2, T], BF16, tag="pB")
                nc.scalar.activation(out=pB, in_=wb, func=Exp)
                # causal mask: keep where q >= k  (k = partition index)
                pmA = ppm.tile([128, 2, T], BF16, tag="pmA", name="pmA")
                nc.gpsimd.affine_select(
                    out=pmA, in_=pA,
                    compare_op=mybir.AluOpType.is_ge,
                    fill=0.0, base=0, channel_multiplier=-1,
                    pattern=[[0, 2], [1, T]],
                )
                pmB = ppm.tile([128, 2, T], BF16, tag="pmB", name="pmB")
                nc.gpsimd.affine_select(
                    out=pmB, in_=pB,
                    compare_op=mybir.AluOpType.is_ge,
                    fill=0.0, base=0, channel_multiplier=-1,
                    pattern=[[0, 2], [1, T]],
                )
                pm_t[P] = (pmA, pmB)

            # o(O) = PmT.T @ v_ext ; col H = softmax denominator
            if Q >= 2:
                O_ = Q - 2
                pmA, pmB = pm_t.pop(O_)
                gb = O_ * QUAD // G
                ob0 = (O_ * QUAD) % G
                if ob0 == 0:
                    osb_t[gb] = posb.tile([128, G, H], F32, tag="o_sb",
                                          name="o_sb")
                o_sb = osb_t[gb]
                for pr in range(QUAD // 2):
                    v_sb = vsb_t.pop((O_, pr))
                    pm = pmA if pr == 0 else pmB
                    o_ps = po.tile([128, 2, 256], F32, tag="o_ps")
                    for jj in range(2):
                        nc.tensor.matmul(o_ps[:, jj, 0 : H + 1], pm[:, jj, :],
                                         v_sb[:, jj, 0 : H + 1],
                                         start=True, stop=True)
                    r = psr.tile([128, 2], F32, tag="r")
                    nc.vector.reciprocal(out=r, in_=o_ps[:, :, H])
                    ob = ob0 + pr * 2
                    nc.vector.tensor_scalar_mul(
                        out=o_sb[:, ob, :], in0=o_ps[:, 0, 0:H],
                        scalar1=r[:, 0:1],
                    )
                    nc.scalar.mul(out=o_sb[:, ob + 1, :], in_=o_ps[:, 1, 0:H],
                                  mul=r[:, 1:2])
                if ob0 + QUAD == G:
                    nc.sync.dma_start(
                        out=o[:, gb * G : (gb + 1) * G, :], in_=o_sb
                    )
                    del osb_t[gb]
    return nc


_cached = {}


def _get_nc(nb):
    if nb not in _cached:
        _cached[nb] = build_nc(nb)
    return _cached[nb]


def prep_inputs(x, Wq, Wk, Wv, nb=NB, ncores=NCORES):
    """Host-side sharding + layout/dtype prep + weight folding + zero-pad."""
    x = np.asarray(x, dtype=np.float32)
    A = (np.asarray(Wq, np.float32).T @ np.asarray(Wk, np.float32)) * SCALE
    a_bf = np.zeros((EP, E), dtype=NPBF16)
    a_bf[0:E] = A.astype(NPBF16)
    wvt_bf = np.zeros((EP, H), dtype=NPBF16)
    wvt_bf[0:E] = np.ascontiguousarray(np.asarray(Wv, np.float32).T).astype(NPBF16)
    in_maps = []
    for c in range(ncores):
        shard = x[c * nb : (c + 1) * nb]                      # [nb, T, E]
        xt = np.zeros((EP, nb * T), dtype=NPBF16)
        xt[0:E] = (
            np.ascontiguousarray(shard.transpose(2, 0, 1))
            .reshape(E, nb * T)
            .astype(NPBF16)
        )
        in_maps.append({"xt": xt, "a": a_bf, "wvt": wvt_bf})
    return in_maps


def kernel(x, Wq, Wk, Wv, _trace=False):
    nc = _get_nc(NB)
    in_maps = prep_inputs(x, Wq, Wk, Wv)
    res = run_bass_kernel_spmd(
        nc, in_maps, core_ids=list(range(NCORES)), trace=_trace
    )
    # o is [T, nb, H] per core; transpose to [nb, T, H] and concat.
    out = np.concatenate(
        [res.results[c]["o"].transpose(1, 0, 2) for c in range(NCORES)], axis=0
    )
    out = np.ascontiguousarray(out, dtype=np.float32)
    if _trace:
        kernel.last_result = res
    return out


# revision 10
# speedup vs baseline: 2.0134x; 1.0055x over previous
"""Trainium2 Bass kernel for nn_Head (single-head causal self-attention).

Module:  q = x@Wq.T, k = x@Wk.T, v = x@Wv.T
         wei = softmax(causal_mask(q@k.T * E**-0.5))
         out = wei @ v
Shapes:  x [2048, 128, 192], Wq/Wk/Wv [192, 192] -> out [2048, 128, 192]

Strategy (pure data parallel over the batch dim, 8 cores x 256 batches):
  - Weight fold: wei = x @ A @ x.T with A = (Wq.T @ Wk) * SCALE, so only one
    projection ("g = x @ A") is needed for the attention logits.
  - Host pads the feature dim E 192 -> 256 with zeros and ships x transposed
    per-core as xt[e, b*T + t] in bf16.  The pad makes every K-chained
    matmul a full K=128 x K=128 pair: half-empty K=64 chain matmuls keep
    the PE's HAM activity monitor below its un-throttle threshold and lock
    the whole kernel at 1.2 GHz.
  - Logits are computed TRANSPOSED (weiT[k, q]) so the masked exp'd tile is
    directly the lhsT of the output matmul -- no PE transposes.
  - weiT's two K-halves are two independent single matmuls into separate
    PSUM banks; exp(wa+wb) = exp(wa)*exp(wb) merges them (ScalarE exp x2 +
    GpSimd mask + DVE multiply) -- single matmuls hide their LDWEIGHTS.
  - g and v keep PSUM accumulation chains but interleave the two chains of
    each quad across banks so chain LDWEIGHTS loads hide under the other
    chain's streaming.
  - Row sums ride along the output matmul via a ones-column on v; ScalarE /
    DVE normalize with per-partition reciprocals.
"""

import os
import sys

sys.path.insert(0, "/opt/trn_rl_repo")

import numpy as np
import ml_dtypes
from contextlib import ExitStack

import json

import concourse.bass as bass
import concourse.bass2jax as bass2jax
import concourse.mybir as mybir
import concourse.tile as tile
from concourse.bass_utils import (
    compile_bir_kernel as _orig_compile_bir_kernel,
    run_bass_kernel_spmd,
)

BF16 = mybir.dt.bfloat16
F32 = mybir.dt.float32
NPBF16 = ml_dtypes.bfloat16

B, T, E, H = 2048, 128, 192, 192
EP = 256                    # zero-padded feature dim (2 x 128)
NCORES = 8
NB = B // NCORES            # batches per core
SCALE = float(E) ** -0.5
G = 8                       # batches per DMA group
QUAD = 4                    # batches per pipeline stage
NGROUPS = NB // G


def _patch_tile_tail_drain():
    """Walrus rejects the TileContext tail Drain when it carries more than a
    couple of sem waits ("Too many sync wait commands").  Redistribute the
    waits onto single-wait SP nops emitted between the drain and barrier."""
    if getattr(tile.TileContext, "_tail_drain_patched", False):
        return

    def _drain_and_barrier(self, tick_clock, wait_clock):
        from concourse.tile import ScopedClock

        drain_inst = self.nc.sync.drain()
        wait_clock.add_sem_waits(
            drain_inst.ins, ScopedClock({None: tick_clock.global_clock})
        )
        waits = list(drain_inst.ins.sync_info.on_wait or [])
        if len(waits) > 1:
            drain_inst.ins.sync_info = mybir.SyncInfo(
                on_wait=[waits[0]], on_update=[]
            )
            for w in waits[1:]:
                nop = self.nc.sync.nop()
                nop.ins.sync_info = mybir.SyncInfo(on_wait=[w], on_update=[])
        self.nc.all_engine_barrier()
        assert self.sems is not None
        popped = self.nc._tile_sem_poison_stack.pop()
        assert popped is self._sem_poison
        self.nc.clear_and_free_semaphores(list(self.sems.allocated().values()))
        self.nc.all_engine_barrier()

    tile.TileContext._drain_and_barrier = _drain_and_barrier
    tile.TileContext._tail_drain_patched = True


def _split_multi_waits(bir_json: bytes) -> bytes:
    """This container's walrus supports only ONE sync-wait slot per
    instruction ("Too many sync wait commands").  Hoist extra waits onto
    single-wait NoOps inserted just before the instruction (same engine, so
    per-engine program order and blocking semantics are preserved)."""
    d = json.loads(bir_json)
    n = 0
    for f in d.get("functions", []):
        for bb in f.get("blocks", []):
            insts = bb.get("instructions", [])
            out = []
            changed = False
            for inst in insts:
                si = inst.get("sync_info")
                waits = (si.get("on_wait") or []) if si else []
                if len(waits) > 1:
                    changed = True
                    for w in waits[:-1]:
                        n += 1
                        out.append({
                            "debug": inst.get("debug"),
                            "engine": inst["engine"],
                            "ins": [],
                            "name": f"WSPLIT-{n}",
                            "opcode": "NoOp",
                            "outs": [],
                            "sync_info": {"on_update": [], "on_wait": [w]},
                        })
                    si["on_wait"] = [waits[-1]]
                out.append(inst)
            if changed:
                bb["instructions"] = out
    if n == 0:
        return bir_json
    return json.dumps(d).encode()


def _patched_compile_bir_kernel(bir_json, tmpdir, neff_name="file.neff"):
    if isinstance(bir_json, str):
        bir_json = bir_json.encode()
    return _orig_compile_bir_kernel(_split_multi_waits(bir_json), tmpdir, neff_name)


bass2jax.compile_bir_kernel = _patched_compile_bir_kernel


def build_nc(nb=NB):
    _patch_tile_tail_drain()
    nc = bass.Bass(trn_type="TRN2")

    xt = nc.dram_tensor("xt", [EP, nb * T], BF16, kind="ExternalInput")
    a = nc.dram_tensor("a", [EP, E], BF16, kind="ExternalInput")
    wvt = nc.dram_tensor("wvt", [EP, H], BF16, kind="ExternalInput")
    # Output laid out [t, b, h] for contiguous DMA; host transposes.
    o = nc.dram_tensor("o", [T, nb, H], BF16, kind="ExternalOutput")

    nq = nb // QUAD
    Exp = mybir.ActivationFunctionType.Exp

    with tile.TileContext(nc) as tc, ExitStack() as ctx:
        singles = ctx.enter_context(tc.tile_pool(name="singles", bufs=1))
        px = ctx.enter_context(tc.tile_pool(name="px", bufs=4))
        pgsb = ctx.enter_context(tc.tile_pool(name="pgsb", bufs=3))
        ppa = ctx.enter_context(tc.tile_pool(name="ppa", bufs=2))
        ppb = ctx.enter_context(tc.tile_pool(name="ppb", bufs=2))
        ppm = ctx.enter_context(tc.tile_pool(name="ppm", bufs=3))
        pvsb = ctx.enter_context(tc.tile_pool(name="pvsb", bufs=6))
        psr = ctx.enter_context(tc.tile_pool(name="psr", bufs=4))
        posb = ctx.enter_context(tc.tile_pool(name="posb", bufs=2))

        pg = ctx.enter_context(tc.tile_pool(name="pg", bufs=1, space="PSUM"))
        pwa = ctx.enter_context(tc.tile_pool(name="pwa", bufs=1, space="PSUM"))
        pwb = ctx.enter_context(tc.tile_pool(name="pwb", bufs=1, space="PSUM"))
        pv = ctx.enter_context(tc.tile_pool(name="pv", bufs=2, space="PSUM"))
        po = ctx.enter_context(tc.tile_pool(name="po", bufs=2, space="PSUM"))

        # Constants: A (lhsT for gT), WvT (rhs for v); hi halves zero-padded
        # to full 128 partitions.
        a_lo = singles.tile([128, E], BF16, tag="a_lo")
        a_hi = singles.tile([128, E], BF16, tag="a_hi")
        nc.sync.dma_start(out=a_lo, in_=a[0:128, :])
        nc.sync.dma_start(out=a_hi, in_=a[128:256, :])
        wvt_lo = singles.tile([128, H], BF16, tag="wvt_lo")
        wvt_hi = singles.tile([128, H], BF16, tag="wvt_hi")
        nc.sync.dma_start(out=wvt_lo, in_=wvt[0:128, :])
        nc.sync.dma_start(out=wvt_hi, in_=wvt[128:256, :])

        # Software pipeline over quads: iteration Q emits
        #   gT(Q), v(Q)  ->  weiT(Q-1) + exp + mask  ->  o(Q-2)
        x_tiles = {}     # group -> (xlo, xhi)
        gsb_t = {}       # Q -> gsb ([128,1024]: gT_lo | gT_hi)
        pm_t = {}        # Q -> masked exp'd weiT (PmT)
        vsb_t = {}       # (Q, pr) -> v_sb pair
        osb_t = {}       # group -> o_sb

        for Q in range(nq + 2):
            if Q < nq:
                g = Q * QUAD // G
                if (Q * QUAD) % G == 0:
                    gcol = g * G * T
                    xlo = px.tile([128, G * T], BF16, tag="xlo")
                    xhi = px.tile([128, G * T], BF16, tag="xhi")
                    nc.sync.dma_start(out=xlo, in_=xt[0:128, gcol : gcol + G * T])
                    nc.sync.dma_start(out=xhi, in_=xt[128:256, gcol : gcol + G * T])
                    x_tiles[g] = (xlo, xhi)
                xlo, xhi = x_tiles[g]
                qs = (Q * QUAD * T) % (G * T)
                qcols = slice(qs, qs + QUAD * T)

                # gT = A.T @ xT for 4 batches; the lo-rows chain (bank0) and
                # hi-rows chain (bank1) are interleaved so each chain's
                # LDWEIGHTS hides under the other chain's streaming.
                gt = pg.tile([128, 1024], F32, tag="gt")
                v_ps0 = pv.tile([128, 2, 256], F32, tag="v_ps", name="v_ps0")
                v_ps1 = pv.tile([128, 2, 256], F32, tag="v_ps", name="v_ps1")
                # g chains woven between v chain halves: every chain-stop
                # LDWEIGHTS gets a preceding stream to hide under.
                nc.tensor.matmul(gt[:, 0:512], a_lo[:, 0:128], xlo[:, qcols],
                                 start=True, stop=False)
                nc.tensor.matmul(gt[0:64, 512:1024], a_lo[:, 128:192],
                                 xlo[:, qcols], start=True, stop=False)
                b0 = qs
                b1 = qs + 2 * T
                nc.tensor.matmul(v_ps0[:, 0, 0:H], xlo[:, b0 : b0 + T],
                                 wvt_lo, start=True, stop=False)
                nc.tensor.matmul(v_ps1[:, 0, 0:H], xlo[:, b1 : b1 + T],
                                 wvt_lo, start=True, stop=False)
                nc.tensor.matmul(gt[:, 0:512], a_hi[:, 0:128], xhi[:, qcols],
                                 start=False, stop=True)
                nc.tensor.matmul(v_ps0[:, 0, 0:H], xhi[:, b0 : b0 + T],
                                 wvt_hi, start=False, stop=True)
                nc.tensor.matmul(gt[0:64, 512:1024], a_hi[:, 128:192],
                                 xhi[:, qcols], start=False, stop=True)
                nc.tensor.matmul(v_ps1[:, 0, 0:H], xhi[:, b1 : b1 + T],
                                 wvt_hi, start=False, stop=True)
                gsb = pgsb.tile([128, 1024], BF16, tag="gsb")
                nc.scalar.copy(out=gsb[:, 0:512], in_=gt[:, 0:512])
                nc.vector.tensor_copy(out=gsb[0:64, 512:1024],
                                      in_=gt[0:64, 512:1024])
                gsb_t[Q] = gsb

                for jj in (1,):
                    b0 = qs + jj * T
                    b1 = qs + (2 + jj) * T
                    nc.tensor.matmul(v_ps0[:, jj, 0:H], xlo[:, b0 : b0 + T],
                                     wvt_lo, start=True, stop=False)
                    nc.tensor.matmul(v_ps1[:, jj, 0:H], xlo[:, b1 : b1 + T],
                                     wvt_lo, start=True, stop=False)
                    nc.tensor.matmul(v_ps0[:, jj, 0:H], xhi[:, b0 : b0 + T],
                                     wvt_hi, start=False, stop=True)
                    nc.tensor.matmul(v_ps1[:, jj, 0:H], xhi[:, b1 : b1 + T],
                                     wvt_hi, start=False, stop=True)
                for pr, v_ps in ((0, v_ps0), (1, v_ps1)):
                    v_sb = pvsb.tile([128, 2, 200], BF16, tag="v_sb")
                    nc.gpsimd.memset(v_sb[:, :, H : H + 1], 1.0)
                    nc.vector.tensor_copy(out=v_sb[:, :, 0:H],
                                          in_=v_ps[:, :, 0:H])
                    vsb_t[(Q, pr)] = v_sb

            # weiT(P) = xT.T @ gT as full-K (padded) chains: batch pair 0
            # in bank A, pair 1 in bank B, chains interleaved across banks
            # so each chain's LDWEIGHTS hides under the other's streaming.
            if 1 <= Q <= nq:
                P = Q - 1
                pg_ = P * QUAD // G
                xlo_p, xhi_p = x_tiles[pg_]
                ps_ = (P * QUAD * T) % (G * T)
                gsb = gsb_t.pop(P)
                if P < 3:
                    # The padded wei chains read gsb rows 64:128 of the hi
                    # half (zero lhsT rows x junk = NaN risk); zero each
                    # pool buffer once.
                    nc.gpsimd.memset(gsb[64:128, 512:1024], 0.0)
                wa = pwa.tile([128, 2, T], F32, tag="wa")
                wb = pwb.tile([128, 2, T], F32, tag="wb")
                for jj in range(2):
                    ka = ps_ + jj * T
                    kb = ps_ + (2 + jj) * T
                    ja = slice(jj * T, (jj + 1) * T)
                    jb = slice((2 + jj) * T, (3 + jj) * T)
                    nc.tensor.matmul(wa[:, jj, :], xlo_p[:, ka : ka + T],
                                     gsb[:, ja], start=True, stop=False)
                    nc.tensor.matmul(wb[:, jj, :], xlo_p[:, kb : kb + T],
                                     gsb[:, jb], start=True, stop=False)
                    nc.tensor.matmul(wa[:, jj, :], xhi_p[:, ka : ka + T],
                                     gsb[:, 512 + jj * T : 512 + (jj + 1) * T],
                                     start=False, stop=True)
                    nc.tensor.matmul(wb[:, jj, :], xhi_p[:, kb : kb + T],
                                     gsb[:, 512 + (2 + jj) * T : 512 + (3 + jj) * T],
                                     start=False, stop=True)
                pA = ppa.tile([128, 2, T], BF16, tag="pA")
                nc.scalar.activation(out=pA, in_=wa, func=Exp)
                pB = ppb.tile([128, 2, T], BF16, tag="pB")
                nc.scalar.activation(out=pB, in_=wb, func=Exp)
                # causal mask: keep where q >= k  (k = partition index)
                pmA = ppm.tile([128, 2, T], BF16, tag="pmA", name="pmA")
                nc.gpsimd.affine_select(
                    out=pmA, in_=pA,
                    compare_op=mybir.AluOpType.is_ge,
                    fill=0.0, base=0, channel_multiplier=-1,
                    pattern=[[0, 2], [1, T]],
                )
                pmB = ppm.tile([128, 2, T], BF16, tag="pmB", name="pmB")
                nc.gpsimd.affine_select(
                    out=pmB, in_=pB,
                    compare_op=mybir.AluOpType.is_ge,
                    fill=0.0, base=0, channel_multiplier=-1,
                    pattern=[[0, 2], [1, T]],
                )
                pm_t[P] = (pmA, pmB)

            # o(O) = PmT.T @ v_ext ; col H = softmax denominator
            if Q >= 2:
                O_ = Q - 2
                pmA, pmB = pm_t.pop(O_)
                gb = O_ * QUAD // G
                ob0 = (O_ * QUAD) % G
                if ob0 == 0:
                    osb_t[gb] = posb.tile([128, G, H], BF16, tag="o_sb",
                                          name="o_sb")
                o_sb = osb_t[gb]
                for pr in range(QUAD // 2):
                    v_sb = vsb_t.pop((O_, pr))
                    pm = pmA if pr == 0 else pmB
                    o_ps = po.tile([128, 2, 256], F32, tag="o_ps")
                    for jj in range(2):
                        nc.tensor.matmul(o_ps[:, jj, 0 : H + 1], pm[:, jj, :],
                                         v_sb[:, jj, 0 : H + 1],
                                         start=True, stop=True)
                    r = psr.tile([128, 2], F32, tag="r")
                    nc.vector.reciprocal(out=r, in_=o_ps[:, :, H])
                    ob = ob0 + pr * 2
                    nc.vector.tensor_scalar_mul(
                        out=o_sb[:, ob, :], in0=o_ps[:, 0, 0:H],
                        scalar1=r[:, 0:1],
                    )
                    nc.scalar.mul(out=o_sb[:, ob + 1, :], in_=o_ps[:, 1, 0:H],
                                  mul=r[:, 1:2])
                if ob0 + QUAD == G:
                    nc.sync.dma_start(
                        out=o[:, gb * G : (gb + 1) * G, :], in_=o_sb
                    )
                    del osb_t[gb]
    return nc


_cached = {}


def _get_nc(nb):
    if nb not in _cached:
        _cached[nb] = build_nc(nb)
    return _cached[nb]


def prep_inputs(x, Wq, Wk, Wv, nb=NB, ncores=NCORES):
    """Host-side sharding + layout/dtype prep + weight folding + zero-pad."""
    x = np.asarray(x, dtype=np.float32)
    A = (np.asarray(Wq, np.float32).T @ np.asarray(Wk, np.float32)) * SCALE
    a_bf = np.zeros((EP, E), dtype=NPBF16)
    a_bf[0:E] = A.astype(NPBF16)
    wvt_bf = np.zeros((EP, H), dtype=NPBF16)
    wvt_bf[0:E] = np.ascontiguousarray(np.asarray(Wv, np.float32).T).astype(NPBF16)
    in_maps = []
    for c in range(ncores):
        shard = x[c * nb : (c + 1) * nb]                      # [nb, T, E]
        xt = np.zeros((EP, nb * T), dtype=NPBF16)
        xt[0:E] = (
            np.ascontiguousarray(shard.transpose(2, 0, 1))
            .reshape(E, nb * T)
            .astype(NPBF16)
        )
        in_maps.append({"xt": xt, "a": a_bf, "wvt": wvt_bf})
    return in_maps


def kernel(x, Wq, Wk, Wv, _trace=False):
    nc = _get_nc(NB)
    in_maps = prep_inputs(x, Wq, Wk, Wv)
    res = run_bass_kernel_spmd(
        nc, in_maps, core_ids=list(range(NCORES)), trace=_trace
    )
    # o is [T, nb, H] per core; transpose to [nb, T, H] and concat.
    out = np.concatenate(
        [np.asarray(res.results[c]["o"], dtype=np.float32).transpose(1, 0, 2)
         for c in range(NCORES)], axis=0
    )
    out = np.ascontiguousarray(out, dtype=np.float32)
    if _trace:
        kernel.last_result = res
    return out
